# revision 20
# baseline (speedup 1.0000x reference)
"""Multi-head attention (B=2, T=2048, F=1024, H=16) on 8 trn2 NeuronCores.

Sharding: tensor-parallel over heads — 2 heads per core. Each core computes
Q^T/K^T/V^T projections for its head pair (column-sliced Wq/Wk/Wv), runs
attention, and a row-sliced output projection producing a partial (B,T,F)
output; the host sums the 8 partials and adds bo.

Layout: everything is computed transposed (Q^T, K^T, V^T, S^T = K Q^T,
ctx^T) so the only on-chip transposes are 16 cheap 128x128 PE transposes
per batch to build token-major V for the PV matmul. A ones-column appended
to V makes the softmax denominator fall out of the PV matmul for free;
normalization is deferred to after PV (it scales matmul columns linearly).

Scheduling: the attention inner loop alternates S-matmul -> exp(ScalarE)
-> PV-matmul, which leaves the PE idle while exp runs. Projection /
output-projection / V-transpose work of the neighbouring batch is emitted
*interleaved* into the attention loop (one slice per t2 step) so the PE's
in-order stream always has independent work during exp waits.

MODE selects the matmul operand dtype for the bulk pipeline:
  "f32r": float32r everywhere (~2^-13 operand rounding) — most accurate.
  "bf16": bfloat16 X/W/Q/K/V/expS (faster PE + half the input DMA);
          softmax-normalization and output projection stay float32r.
"""

import os
from collections import deque

import numpy as np

import concourse.mybir as mybir
import concourse.tile as tile
from concourse import bacc
from concourse.bass_utils import run_bass_kernel_spmd

B, T, F = 2, 2048, 1024
H, DK = 16, 64
NCORES = 8
HPC = H // NCORES          # heads per core
HD = HPC * DK              # 128 head dims per core
KT_ = F // 128             # 8 contraction tiles for projections
TW = 1024                  # t1 window (exp free-dim)
NW = T // TW               # 2 windows
NT2 = T // 128             # 16 t2 tiles

f32 = mybir.dt.float32
f32r = mybir.dt.float32r
bf16 = mybir.dt.bfloat16
EXP = mybir.ActivationFunctionType.Exp
MULT = mybir.AluOpType.mult

MODE = os.environ.get("MHA_MODE", "f32r")


def build_nc(include_bias: bool, mode: str = MODE):
    mdt = bf16 if mode == "bf16" else f32r
    nc = bacc.Bacc("TRN2", target_bir_lowering=False)

    xqT = nc.dram_tensor("xqT", [B, F, T], mdt, kind="ExternalInput")
    xkT = nc.dram_tensor("xkT", [B, F, T], mdt, kind="ExternalInput")
    xvT = nc.dram_tensor("xvT", [B, F, T], mdt, kind="ExternalInput")
    wq = nc.dram_tensor("wq", [F, HD], mdt, kind="ExternalInput")
    wk = nc.dram_tensor("wk", [F, HD], mdt, kind="ExternalInput")
    wv = nc.dram_tensor("wv", [F, HD], mdt, kind="ExternalInput")
    wo = nc.dram_tensor("wo", [HD, F], f32r, kind="ExternalInput")
    ident_in = nc.dram_tensor("ident", [128, 128], mdt, kind="ExternalInput")
    # sel[:, c*64:(c+1)*64] = e_c selector (row c ones) for the recip bcast
    sel_in = nc.dram_tensor("sel", [8, 8 * 64], f32r, kind="ExternalInput")
    identr_in = nc.dram_tensor("identr", [128, 128], f32, kind="ExternalInput")
    if include_bias:
        bq = nc.dram_tensor("bq", [1, HD], mdt, kind="ExternalInput")
        bk = nc.dram_tensor("bk", [1, HD], mdt, kind="ExternalInput")
        bv = nc.dram_tensor("bv", [1, HD], mdt, kind="ExternalInput")
    out = nc.dram_tensor("out", [B, T, F], f32, kind="ExternalOutput")

    with tile.TileContext(nc) as tc:
        with (
            tc.tile_pool(name="const", bufs=1) as cpool,
            tc.tile_pool(name="xs", bufs=8) as xpool,
            tc.tile_pool(name="work", bufs=1) as wpool,
            tc.tile_pool(name="psum", bufs=1, space="PSUM") as psum,
        ):
            # ---- constants / weights resident in SBUF ----
            wq_s = cpool.tile([128, KT_, HD], mdt, tag="wq")
            wk_s = cpool.tile([128, KT_, HD], mdt, tag="wk")
            wv_s = cpool.tile([128, KT_, HD], mdt, tag="wv")
            wo_s = cpool.tile([HD, F], f32r, tag="wo")
            ident = cpool.tile([128, 128], mdt, tag="ident")
            sel = cpool.tile([8, 8 * 64], f32r, tag="sel")
            identr = cpool.tile([128, 128], f32, tag="identr")
            nc.sync.dma_start(sel[:], sel_in[:])
            nc.sync.dma_start(identr[:], identr_in[:])
            nc.sync.dma_start(wq_s[:], wq.rearrange("(k p) m -> p k m", p=128))
            nc.sync.dma_start(wk_s[:], wk.rearrange("(k p) m -> p k m", p=128))
            nc.sync.dma_start(wv_s[:], wv.rearrange("(k p) m -> p k m", p=128))
            nc.sync.dma_start(wo_s[:], wo[:])
            nc.sync.dma_start(ident[:], ident_in[:])

            with nc.allow_low_precision(reason="matmul operand rounding"):
                # [1, 64] of ones: stationary for the 1/sum broadcast matmul
                ones64_f = wpool.tile([1, 64], f32, tag="c_f")
                nc.vector.memset(ones64_f[:], 1.0)
                ones64 = cpool.tile([1, 64], f32r, tag="ones64")
                nc.vector.tensor_copy(ones64[:], ones64_f[:])
                # ones column pair for V1 (written into cols 64 and 129)
                onescol_f = wpool.tile([128, 2], f32, tag="c_f2")
                nc.vector.memset(onescol_f[:], 1.0)
                onescol = cpool.tile([128, 2], mdt, tag="onescol")
                nc.vector.tensor_copy(onescol[:], onescol_f[:])
                if include_bias:
                    bq_s = cpool.tile([1, HD], mdt, tag="bq")
                    bk_s = cpool.tile([1, HD], mdt, tag="bk")
                    bv_s = cpool.tile([1, HD], mdt, tag="bv")
                    nc.sync.dma_start(bq_s[:], bq[:])
                    nc.sync.dma_start(bk_s[:], bk[:])
                    nc.sync.dma_start(bv_s[:], bv[:])
                    onesrow_f = wpool.tile([1, 512], f32, tag="c_f3")
                    nc.vector.memset(onesrow_f[:], 1.0)
                    onesrow = cpool.tile([1, 512], mdt, tag="onesrow")
                    nc.vector.tensor_copy(onesrow[:], onesrow_f[:])

            # per-batch persistent tiles
            qt = {}; kt = {}; vt = {}; v1 = {}; ctxT = {}
            for b in range(B):
                qt[b] = wpool.tile([HD, T], mdt, tag="qt", bufs=2, name=f"qt{b}")
                kt[b] = wpool.tile([HD, T], mdt, tag="kt", bufs=2, name=f"kt{b}")
                vt[b] = wpool.tile([HD, T], mdt, tag="vt", bufs=2, name=f"vt{b}")
                v1[b] = wpool.tile([128, NT2, 2 * 65], mdt, tag="v1", bufs=2,
                                   name=f"v1{b}")
                ctxT[b] = wpool.tile([HD, T], f32r, tag="ctxT", bufs=2,
                                     name=f"ctxT{b}")

            def gen_proj(b):
                """Q^T/K^T/V^T projections for batch b. Yields at item
                boundaries so it can be interleaved into attention."""
                for (xsrc, w_s, dst, bias) in (
                    (xqT, wq_s, qt[b], "q"),
                    (xkT, wk_s, kt[b], "k"),
                    (xvT, wv_s, vt[b], "v"),
                ):
                    xts = []
                    for k in range(KT_):
                        xt = xpool.tile([128, T], mdt, tag="xt")
                        nc.sync.dma_start(xt[:], xsrc[b, k * 128:(k + 1) * 128, :])
                        xts.append(xt)
                    yield
                    for n in range(T // 512):
                        ps = psum.tile([128, 512], f32, tag="pa", bufs=2)
                        sl = slice(n * 512, (n + 1) * 512)
                        for k in range(KT_):
                            nc.tensor.matmul(
                                ps[:], w_s[:, k, :], xts[k][:, sl],
                                start=(k == 0),
                                stop=(k == KT_ - 1) and not include_bias,
                            )
                            if k == 3:
                                yield
                        if include_bias:
                            bsrc = {"q": bq_s, "k": bk_s, "v": bv_s}[bias]
                            nc.tensor.matmul(ps[:], bsrc[:], onesrow[:],
                                             start=False, stop=True)
                        with nc.allow_low_precision(reason="rounding"):
                            nc.vector.tensor_copy(dst[:, sl], ps[:])
                        yield

            def gen_v1(b):
                """Token-major V (+ones cols) via PE transposes of V^T."""
                for tcid in range(NT2):
                    pt = psum.tile([128, 128], mdt, tag="pa", bufs=2)
                    tsl = slice(tcid * 128, (tcid + 1) * 128)
                    nc.tensor.transpose(pt[:], vt[b][:, tsl], ident[:])
                    with nc.allow_low_precision(reason="rounding"):
                        nc.vector.tensor_copy(v1[b][:, tcid, 0:64], pt[:, 0:64])
                        nc.vector.tensor_copy(v1[b][:, tcid, 65:129],
                                              pt[:, 64:128])
                        nc.vector.tensor_copy(v1[b][:, tcid, 64:130:65],
                                              onescol[:])
                    if tcid % 2 == 1:
                        yield

            def gen_oproj(b, lo, hi):
                """Output projection token-chunks [lo, hi) for batch b."""
                for tcid in range(lo, hi):
                    tsl = slice(tcid * 128, (tcid + 1) * 128)
                    ob = wpool.tile([128, F], f32, tag="ob", bufs=2)
                    for half in range(2):
                        po = psum.tile([128, 512], f32, tag="pa", bufs=2)
                        fsl = slice(half * 512, (half + 1) * 512)
                        nc.tensor.matmul(po[:], ctxT[b][:, tsl], wo_s[:, fsl],
                                         start=True, stop=True)
                        nc.vector.tensor_copy(ob[:, fsl], po[:])
                    nc.sync.dma_start(out[b, tsl, :], ob[:])
                    yield

            pending = deque()

            def consume():
                while pending:
                    try:
                        next(pending[0])
                        return
                    except StopIteration:
                        pending.popleft()

            def attn_window(b, n):
                wsl = slice(n * TW, (n + 1) * TW)
                for h in range(HPC):
                    hsl = slice(h * 64, (h + 1) * 64)
                    vsl = slice(h * 65, (h + 1) * 65)
                    tp = (h * 64, 0)
                    ctx = psum.tile([65, TW], f32, tag="ctx", bufs=1)
                    for t2 in range(NT2):
                        t2sl = slice(t2 * 128, (t2 + 1) * 128)
                        s = psum.tile([128, TW], f32, tag="st", bufs=2)
                        es = wpool.tile([128, TW], mdt, tag="es",
                                        bufs=(4 if mdt == bf16 else 3))
                        for q in range(TW // 512):
                            qsl = slice(n * TW + q * 512, n * TW + (q + 1) * 512)
                            nc.tensor.matmul(s[:, q * 512:(q + 1) * 512],
                                             kt[b][hsl, t2sl], qt[b][hsl, qsl],
                                             start=True, stop=True,
                                             tile_position=tp)
                        with nc.allow_low_precision(reason="rounding"):
                            nc.scalar.activation(es[:], s[:], EXP, scale=0.125)
                        consume()
                        for q in range(TW // 512):
                            csl = slice(q * 512, (q + 1) * 512)
                            nc.tensor.matmul(ctx[:, csl], v1[b][:, t2, vsl],
                                             es[:, csl],
                                             start=(t2 == 0),
                                             stop=(t2 == NT2 - 1))
                    # normalize: ctxT[hsl, wsl] = ctx[0:64] / sums.
                    # The sums row is transposed into columns with 8 tiny PE
                    # transposes so the reciprocal runs on 128 lanes instead
                    # of one, then transposed back and broadcast via selector
                    # matmuls.
                    rc = wpool.tile([1, TW], f32, tag="rc", bufs=2)
                    nc.vector.tensor_copy(rc[:], ctx[64:65, :])
                    consume()
                    pts = psum.tile([128, 8], f32, tag="pa", bufs=2)
                    for c in range(8):
                        nc.tensor.transpose(pts[:, c:c + 1],
                                            rc[0:1, c * 128:(c + 1) * 128],
                                            identr[0:1, 0:1])
                    rcc = wpool.tile([128, 8], f32, tag="rcc", bufs=2)
                    nc.vector.reciprocal(rcc[:], pts[:])
                    consume()
                    pr = psum.tile([8, 128], f32, tag="pa", bufs=2)
                    nc.tensor.transpose(pr[:], rcc[:], identr[:])
                    rcr = wpool.tile([8, 128], f32r, tag="rcr", bufs=2)
                    with nc.allow_low_precision(reason="rounding"):
                        nc.vector.tensor_copy(rcr[:], pr[:])
                    consume()
                    scp = psum.tile([64, TW], f32, tag="st", bufs=2)
                    for c in range(8):
                        nc.tensor.matmul(scp[:, c * 128:(c + 1) * 128],
                                         sel[:, c * 64:(c + 1) * 64], rcr[:],
                                         start=True, stop=True)
                    sc = wpool.tile([64, TW], f32, tag="sc", bufs=2)
                    nc.vector.tensor_copy(sc[:], scp[:])
                    consume()
                    with nc.allow_low_precision(reason="rounding"):
                        nc.vector.tensor_tensor(ctxT[b][hsl, wsl], ctx[0:64, :],
                                                sc[:], MULT)

            # batch 0 projections run directly (nothing to hide behind)
            for _ in gen_proj(0):
                pass
            for _ in gen_v1(0):
                pass
            # batch 1 projection work interleaves into batch-0 attention
            pending.append(gen_proj(1))
            pending.append(gen_v1(1))
            attn_window(0, 0)
            pending.append(gen_oproj(0, 0, 8))
            attn_window(0, 1)
            pending.append(gen_oproj(0, 8, 16))
            attn_window(1, 0)
            pending.append(gen_oproj(1, 0, 8))
            attn_window(1, 1)
            pending.append(gen_oproj(1, 8, 16))
            while pending:
                try:
                    next(pending[0])
                except StopIteration:
                    pending.popleft()

    nc.compile()
    return nc


_CACHE = {}


def _get_nc(include_bias: bool):
    key = (include_bias, MODE)
    if key not in _CACHE:
        _CACHE[key] = build_nc(include_bias)
    return _CACHE[key]


def _reference_fallback(query, key_, value, mask, Wq, bq, Wk, bk, Wv, bv, Wo, bo):
    """Plain numpy fallback (only used if the mask is not all-ones)."""
    q = (query @ Wq + bq).reshape(B, T, H, DK).transpose(0, 2, 1, 3)
    k = (key_ @ Wk + bk).reshape(B, T, H, DK).transpose(0, 2, 1, 3)
    v = (value @ Wv + bv).reshape(B, T, H, DK).transpose(0, 2, 1, 3)
    scores = np.einsum("bhqd,bhkd->bhqk", q, k) / np.sqrt(np.float32(DK))
    scores = np.where(mask[:, None, :, :] > 0, scores,
                      np.float32(-10000.0)).astype(np.float32)
    scores -= scores.max(axis=-1, keepdims=True)
    e = np.exp(scores)
    attn = e / e.sum(axis=-1, keepdims=True)
    x = np.einsum("bhqk,bhkd->bhqd", attn, v)
    x = x.transpose(0, 2, 1, 3).reshape(B, T, F)
    return (x @ Wo + bo).astype(np.float32)


def _mdt_np(arr):
    if MODE == "bf16":
        import ml_dtypes
        return np.ascontiguousarray(arr).astype(ml_dtypes.bfloat16)
    return np.ascontiguousarray(arr)


def make_in_maps(query, key_, value, Wq, Wk, Wv, Wo, bq=None, bk=None, bv=None):
    xqT = _mdt_np(query.transpose(0, 2, 1))
    xkT = _mdt_np(key_.transpose(0, 2, 1))
    xvT = _mdt_np(value.transpose(0, 2, 1))
    ident = _mdt_np(np.eye(128, dtype=np.float32))
    identr = np.eye(128, dtype=np.float32)
    sel = np.zeros((8, 8 * 64), np.float32)
    for c in range(8):
        sel[c, c * 64:(c + 1) * 64] = 1.0
    in_maps = []
    for c in range(NCORES):
        csl = slice(c * HD, (c + 1) * HD)
        m = {
            "xqT": xqT, "xkT": xkT, "xvT": xvT, "ident": ident,
            "sel": sel, "identr": identr,
            "wq": _mdt_np(Wq[:, csl]),
            "wk": _mdt_np(Wk[:, csl]),
            "wv": _mdt_np(Wv[:, csl]),
            "wo": np.ascontiguousarray(Wo[csl, :]),
        }
        if bq is not None:
            m["bq"] = _mdt_np(bq[None, csl])
            m["bk"] = _mdt_np(bk[None, csl])
            m["bv"] = _mdt_np(bv[None, csl])
        in_maps.append(m)
    return in_maps


def kernel(**inputs) -> np.ndarray:
    query = np.asarray(inputs["query"], np.float32)
    key_ = np.asarray(inputs.get("key_", inputs.get("key")), np.float32)
    value = np.asarray(inputs["value"], np.float32)
    mask = np.asarray(inputs["mask"])
    Wq, bq = np.asarray(inputs["Wq"], np.float32), np.asarray(inputs["bq"], np.float32)
    Wk, bk = np.asarray(inputs["Wk"], np.float32), np.asarray(inputs["bk"], np.float32)
    Wv, bv = np.asarray(inputs["Wv"], np.float32), np.asarray(inputs["bv"], np.float32)
    Wo, bo = np.asarray(inputs["Wo"], np.float32), np.asarray(inputs["bo"], np.float32)

    if not (mask > 0).all():
        return _reference_fallback(query, key_, value, mask,
                                   Wq, bq, Wk, bk, Wv, bv, Wo, bo)

    include_bias = bool(np.any(bq) or np.any(bk) or np.any(bv))
    nc = _get_nc(include_bias)
    if include_bias:
        in_maps = make_in_maps(query, key_, value, Wq, Wk, Wv, Wo, bq, bk, bv)
    else:
        in_maps = make_in_maps(query, key_, value, Wq, Wk, Wv, Wo)

    res = run_bass_kernel_spmd(nc, in_maps, core_ids=list(range(NCORES)))
    total = res.results[0]["out"]
    for c in range(1, NCORES):
        total = total + res.results[c]["out"]
    return (total + bo).astype(np.float32)


# revision 22
# speedup vs baseline: 1.2510x; 1.2510x over previous
"""Multi-head attention (B=2, T=2048, F=1024, H=16) on 8 trn2 NeuronCores.

Sharding: tensor-parallel over heads — 2 heads per core. Each core computes
Q^T/K^T/V^T projections for its head pair (column-sliced Wq/Wk/Wv), runs
attention, and a row-sliced output projection producing a partial (B,T,F)
output; the host sums the 8 partials and adds bo.

Layout: everything is computed transposed (Q^T, K^T, V^T, S^T = K Q^T,
ctx^T) so the only on-chip transposes are 16 cheap 128x128 PE transposes
per batch to build token-major V for the PV matmul. A ones-column appended
to V makes the softmax denominator fall out of the PV matmul for free;
normalization is deferred to after PV (it scales matmul columns linearly).

Scheduling: the attention inner loop alternates S-matmul -> exp(ScalarE)
-> PV-matmul, which leaves the PE idle while exp runs. Projection /
output-projection / V-transpose work of the neighbouring batch is emitted
*interleaved* into the attention loop (one slice per t2 step) so the PE's
in-order stream always has independent work during exp waits.

MODE selects the matmul operand dtype for the bulk pipeline:
  "f32r": float32r everywhere (~2^-13 operand rounding) — most accurate.
  "bf16": bfloat16 X/W/Q/K/V/expS (faster PE + half the input DMA);
          softmax-normalization and output projection stay float32r.
"""

import os
from collections import deque

import numpy as np

import concourse.mybir as mybir
import concourse.tile as tile
from concourse import bacc
from concourse.bass_utils import run_bass_kernel_spmd

B, T, F = 2, 2048, 1024
H, DK = 16, 64
NCORES = 8
HPC = H // NCORES          # heads per core
HD = HPC * DK              # 128 head dims per core
KT_ = F // 128             # 8 contraction tiles for projections
TW = 512                   # t1 window (exp free-dim, one psum bank)
NW = T // TW               # 2 windows
NT2 = T // 128             # 16 t2 tiles

f32 = mybir.dt.float32
f32r = mybir.dt.float32r
bf16 = mybir.dt.bfloat16
EXP = mybir.ActivationFunctionType.Exp
MULT = mybir.AluOpType.mult

MODE = os.environ.get("MHA_MODE", "f32r")


def build_nc(include_bias: bool, mode: str = MODE):
    mdt = bf16 if mode == "bf16" else f32r
    nc = bacc.Bacc("TRN2", target_bir_lowering=False)

    xqT = nc.dram_tensor("xqT", [B, F, T], mdt, kind="ExternalInput")
    xkT = nc.dram_tensor("xkT", [B, F, T], mdt, kind="ExternalInput")
    xvT = nc.dram_tensor("xvT", [B, F, T], mdt, kind="ExternalInput")
    wq = nc.dram_tensor("wq", [F, HD], mdt, kind="ExternalInput")
    wk = nc.dram_tensor("wk", [F, HD], mdt, kind="ExternalInput")
    wv = nc.dram_tensor("wv", [F, HD], mdt, kind="ExternalInput")
    wo = nc.dram_tensor("wo", [HD, F], f32r, kind="ExternalInput")
    ident_in = nc.dram_tensor("ident", [128, 128], mdt, kind="ExternalInput")
    # sel[:, c*64:(c+1)*64] = e_c selector (row c ones) for the recip bcast
    sel_in = nc.dram_tensor("sel", [8, 8 * 64], f32r, kind="ExternalInput")
    identr_in = nc.dram_tensor("identr", [128, 128], f32, kind="ExternalInput")
    if include_bias:
        bq = nc.dram_tensor("bq", [1, HD], mdt, kind="ExternalInput")
        bk = nc.dram_tensor("bk", [1, HD], mdt, kind="ExternalInput")
        bv = nc.dram_tensor("bv", [1, HD], mdt, kind="ExternalInput")
    out = nc.dram_tensor("out", [B, T, F], f32, kind="ExternalOutput")

    with tile.TileContext(nc) as tc:
        with (
            tc.tile_pool(name="const", bufs=1) as cpool,
            tc.tile_pool(name="xs", bufs=8) as xpool,
            tc.tile_pool(name="work", bufs=1) as wpool,
            tc.tile_pool(name="psum", bufs=1, space="PSUM") as psum,
        ):
            # ---- constants / weights resident in SBUF ----
            wq_s = cpool.tile([128, KT_, HD], mdt, tag="wq")
            wk_s = cpool.tile([128, KT_, HD], mdt, tag="wk")
            wv_s = cpool.tile([128, KT_, HD], mdt, tag="wv")
            wo_s = cpool.tile([HD, F], f32r, tag="wo")
            ident = cpool.tile([128, 128], mdt, tag="ident")
            sel = cpool.tile([8, 8 * 64], f32r, tag="sel")
            identr = cpool.tile([128, 128], f32, tag="identr")
            nc.sync.dma_start(sel[:], sel_in[:])
            nc.sync.dma_start(identr[:], identr_in[:])
            nc.sync.dma_start(wq_s[:], wq.rearrange("(k p) m -> p k m", p=128))
            nc.sync.dma_start(wk_s[:], wk.rearrange("(k p) m -> p k m", p=128))
            nc.sync.dma_start(wv_s[:], wv.rearrange("(k p) m -> p k m", p=128))
            nc.sync.dma_start(wo_s[:], wo[:])
            nc.sync.dma_start(ident[:], ident_in[:])

            with nc.allow_low_precision(reason="matmul operand rounding"):
                # [1, 64] of ones: stationary for the 1/sum broadcast matmul
                ones64_f = wpool.tile([1, 64], f32, tag="c_f")
                nc.vector.memset(ones64_f[:], 1.0)
                ones64 = cpool.tile([1, 64], f32r, tag="ones64")
                nc.vector.tensor_copy(ones64[:], ones64_f[:])
                # ones column pair for V1 (written into cols 64 and 129)
                onescol_f = wpool.tile([128, 2], f32, tag="c_f2")
                nc.vector.memset(onescol_f[:], 1.0)
                onescol = cpool.tile([128, 2], mdt, tag="onescol")
                nc.vector.tensor_copy(onescol[:], onescol_f[:])
                if include_bias:
                    bq_s = cpool.tile([1, HD], mdt, tag="bq")
                    bk_s = cpool.tile([1, HD], mdt, tag="bk")
                    bv_s = cpool.tile([1, HD], mdt, tag="bv")
                    nc.sync.dma_start(bq_s[:], bq[:])
                    nc.sync.dma_start(bk_s[:], bk[:])
                    nc.sync.dma_start(bv_s[:], bv[:])
                    onesrow_f = wpool.tile([1, 512], f32, tag="c_f3")
                    nc.vector.memset(onesrow_f[:], 1.0)
                    onesrow = cpool.tile([1, 512], mdt, tag="onesrow")
                    nc.vector.tensor_copy(onesrow[:], onesrow_f[:])

            # per-batch persistent tiles
            qt = {}; kt = {}; vt = {}; v1 = {}; ctxT = {}
            for b in range(B):
                qt[b] = wpool.tile([HD, T], mdt, tag="qt", bufs=2, name=f"qt{b}")
                kt[b] = wpool.tile([HD, T], mdt, tag="kt", bufs=2, name=f"kt{b}")
                vt[b] = wpool.tile([HD, T], mdt, tag="vt", bufs=2, name=f"vt{b}")
                v1[b] = wpool.tile([128, NT2, 2 * 65], mdt, tag="v1", bufs=2,
                                   name=f"v1{b}")
                ctxT[b] = wpool.tile([HD, T], f32r, tag="ctxT", bufs=2,
                                     name=f"ctxT{b}")

            def gen_proj(b):
                """Q^T/K^T/V^T projections for batch b. Yields at item
                boundaries so it can be interleaved into attention."""
                for (xsrc, w_s, dst, bias) in (
                    (xqT, wq_s, qt[b], "q"),
                    (xkT, wk_s, kt[b], "k"),
                    (xvT, wv_s, vt[b], "v"),
                ):
                    xts = []
                    for k in range(KT_):
                        xt = xpool.tile([128, T], mdt, tag="xt")
                        nc.sync.dma_start(xt[:], xsrc[b, k * 128:(k + 1) * 128, :])
                        xts.append(xt)
                    yield
                    for n in range(T // 512):
                        ps = psum.tile([128, 512], f32, tag="pa", bufs=2)
                        sl = slice(n * 512, (n + 1) * 512)
                        for k in range(KT_):
                            nc.tensor.matmul(
                                ps[:], w_s[:, k, :], xts[k][:, sl],
                                start=(k == 0),
                                stop=(k == KT_ - 1) and not include_bias,
                            )
                            if k == 3:
                                yield
                        if include_bias:
                            bsrc = {"q": bq_s, "k": bk_s, "v": bv_s}[bias]
                            nc.tensor.matmul(ps[:], bsrc[:], onesrow[:],
                                             start=False, stop=True)
                        with nc.allow_low_precision(reason="rounding"):
                            nc.vector.tensor_copy(dst[:, sl], ps[:])
                        yield

            def gen_v1(b):
                """Token-major V (+ones cols) via PE transposes of V^T."""
                for tcid in range(NT2):
                    pt = psum.tile([128, 128], mdt, tag="pa", bufs=2)
                    tsl = slice(tcid * 128, (tcid + 1) * 128)
                    nc.tensor.transpose(pt[:], vt[b][:, tsl], ident[:])
                    with nc.allow_low_precision(reason="rounding"):
                        nc.vector.tensor_copy(v1[b][:, tcid, 0:64], pt[:, 0:64])
                        nc.vector.tensor_copy(v1[b][:, tcid, 65:129],
                                              pt[:, 64:128])
                        nc.vector.tensor_copy(v1[b][:, tcid, 64:130:65],
                                              onescol[:])
                    if tcid % 2 == 1:
                        yield

            def gen_oproj(b, lo, hi):
                """Output projection token-chunks [lo, hi) for batch b."""
                for tcid in range(lo, hi):
                    tsl = slice(tcid * 128, (tcid + 1) * 128)
                    ob = wpool.tile([128, F], f32, tag="ob", bufs=2)
                    for half in range(2):
                        po = psum.tile([128, 512], f32, tag="pa", bufs=2)
                        fsl = slice(half * 512, (half + 1) * 512)
                        nc.tensor.matmul(po[:], ctxT[b][:, tsl], wo_s[:, fsl],
                                         start=True, stop=True)
                        nc.vector.tensor_copy(ob[:, fsl], po[:])
                    nc.sync.dma_start(out[b, tsl, :], ob[:])
                    yield

            pending = deque()

            def consume():
                while pending:
                    try:
                        next(pending[0])
                        return
                    except StopIteration:
                        pending.popleft()

            def norm(b, n, h, ctx):
                """ctxT[hsl, wsl] = ctx[0:64] / sums (sums = ctx row 64).
                The sums row is transposed into columns with tiny PE
                transposes so the reciprocal runs across lanes instead of
                one, then transposed back and broadcast via selector
                matmuls."""
                nch = TW // 128  # 128-col chunks in the window
                wsl = slice(n * TW, (n + 1) * TW)
                hsl = slice(h * 64, (h + 1) * 64)
                rc = wpool.tile([1, TW], f32, tag="rc", bufs=2)
                nc.vector.tensor_copy(rc[:], ctx[64:65, :])
                consume()
                pts = psum.tile([128, nch], f32, tag="pa", bufs=2)
                for c in range(nch):
                    nc.tensor.transpose(pts[:, c:c + 1],
                                        rc[0:1, c * 128:(c + 1) * 128],
                                        identr[0:1, 0:1])
                rcc = wpool.tile([128, nch], f32, tag="rcc", bufs=2)
                nc.vector.reciprocal(rcc[:], pts[:])
                consume()
                pr = psum.tile([nch, 128], f32, tag="pa", bufs=2)
                nc.tensor.transpose(pr[:], rcc[:], identr[:])
                rcr = wpool.tile([nch, 128], f32r, tag="rcr", bufs=2)
                with nc.allow_low_precision(reason="rounding"):
                    nc.vector.tensor_copy(rcr[:], pr[:])
                consume()
                scp = psum.tile([64, TW], f32, tag="st", bufs=4)
                for c in range(nch):
                    nc.tensor.matmul(scp[:, c * 128:(c + 1) * 128],
                                     sel[0:nch, c * 64:(c + 1) * 64], rcr[:],
                                     start=True, stop=True)
                sc = wpool.tile([64, TW], f32, tag="sc", bufs=2)
                nc.vector.tensor_copy(sc[:], scp[:])
                consume()
                with nc.allow_low_precision(reason="rounding"):
                    nc.vector.tensor_tensor(ctxT[b][hsl, wsl], ctx[0:64, :],
                                            sc[:], MULT)

            def attn_window(b, n):
                # both heads advance together so the PE always has the other
                # head's independent matmuls during exp latency
                qsl = slice(n * TW, (n + 1) * TW)
                ctx0 = psum.tile([65, TW], f32, tag="ctx", bufs=2, name="ctx0")
                ctx1 = psum.tile([65, TW], f32, tag="ctx", bufs=2, name="ctx1")
                ctxs = (ctx0, ctx1)
                esb = 6 if mdt == bf16 else 4
                for t2 in range(NT2):
                    t2sl = slice(t2 * 128, (t2 + 1) * 128)
                    ss = []
                    ess = []
                    for h in range(HPC):
                        hsl = slice(h * 64, (h + 1) * 64)
                        s = psum.tile([128, TW], f32, tag="st", bufs=4,
                                      name=f"s{h}")
                        nc.tensor.matmul(s[:], kt[b][hsl, t2sl], qt[b][hsl, qsl],
                                         start=True, stop=True,
                                         tile_position=(h * 64, 0))
                        ss.append(s)
                    for h in range(HPC):
                        es = wpool.tile([128, TW], mdt, tag="es", bufs=esb,
                                        name=f"es{h}")
                        with nc.allow_low_precision(reason="rounding"):
                            nc.scalar.activation(es[:], ss[h][:], EXP,
                                                 scale=0.125)
                        ess.append(es)
                    consume()
                    for h in range(HPC):
                        vsl = slice(h * 65, (h + 1) * 65)
                        nc.tensor.matmul(ctxs[h][:], v1[b][:, t2, vsl],
                                         ess[h][:],
                                         start=(t2 == 0),
                                         stop=(t2 == NT2 - 1))
                for h in range(HPC):
                    norm(b, n, h, ctxs[h])

            # batch 0 projections run directly (nothing to hide behind)
            for _ in gen_proj(0):
                pass
            for _ in gen_v1(0):
                pass
            # batch 1 projection work interleaves into batch-0 attention
            pending.append(gen_proj(1))
            pending.append(gen_v1(1))
            tc_per_w = T // 128 // NW
            for b in range(B):
                for n in range(NW):
                    attn_window(b, n)
                    pending.append(gen_oproj(b, n * tc_per_w, (n + 1) * tc_per_w))
            while pending:
                try:
                    next(pending[0])
                except StopIteration:
                    pending.popleft()

    nc.compile()
    return nc


_CACHE = {}


def _get_nc(include_bias: bool):
    key = (include_bias, MODE)
    if key not in _CACHE:
        _CACHE[key] = build_nc(include_bias)
    return _CACHE[key]


def _reference_fallback(query, key_, value, mask, Wq, bq, Wk, bk, Wv, bv, Wo, bo):
    """Plain numpy fallback (only used if the mask is not all-ones)."""
    q = (query @ Wq + bq).reshape(B, T, H, DK).transpose(0, 2, 1, 3)
    k = (key_ @ Wk + bk).reshape(B, T, H, DK).transpose(0, 2, 1, 3)
    v = (value @ Wv + bv).reshape(B, T, H, DK).transpose(0, 2, 1, 3)
    scores = np.einsum("bhqd,bhkd->bhqk", q, k) / np.sqrt(np.float32(DK))
    scores = np.where(mask[:, None, :, :] > 0, scores,
                      np.float32(-10000.0)).astype(np.float32)
    scores -= scores.max(axis=-1, keepdims=True)
    e = np.exp(scores)
    attn = e / e.sum(axis=-1, keepdims=True)
    x = np.einsum("bhqk,bhkd->bhqd", attn, v)
    x = x.transpose(0, 2, 1, 3).reshape(B, T, F)
    return (x @ Wo + bo).astype(np.float32)


def _mdt_np(arr):
    if MODE == "bf16":
        import ml_dtypes
        return np.ascontiguousarray(arr).astype(ml_dtypes.bfloat16)
    return np.ascontiguousarray(arr)


def make_in_maps(query, key_, value, Wq, Wk, Wv, Wo, bq=None, bk=None, bv=None):
    xqT = _mdt_np(query.transpose(0, 2, 1))
    xkT = _mdt_np(key_.transpose(0, 2, 1))
    xvT = _mdt_np(value.transpose(0, 2, 1))
    ident = _mdt_np(np.eye(128, dtype=np.float32))
    identr = np.eye(128, dtype=np.float32)
    sel = np.zeros((8, 8 * 64), np.float32)
    for c in range(8):
        sel[c, c * 64:(c + 1) * 64] = 1.0
    in_maps = []
    for c in range(NCORES):
        csl = slice(c * HD, (c + 1) * HD)
        m = {
            "xqT": xqT, "xkT": xkT, "xvT": xvT, "ident": ident,
            "sel": sel, "identr": identr,
            "wq": _mdt_np(Wq[:, csl]),
            "wk": _mdt_np(Wk[:, csl]),
            "wv": _mdt_np(Wv[:, csl]),
            "wo": np.ascontiguousarray(Wo[csl, :]),
        }
        if bq is not None:
            m["bq"] = _mdt_np(bq[None, csl])
            m["bk"] = _mdt_np(bk[None, csl])
            m["bv"] = _mdt_np(bv[None, csl])
        in_maps.append(m)
    return in_maps


def kernel(**inputs) -> np.ndarray:
    query = np.asarray(inputs["query"], np.float32)
    key_ = np.asarray(inputs.get("key_", inputs.get("key")), np.float32)
    value = np.asarray(inputs["value"], np.float32)
    mask = np.asarray(inputs["mask"])
    Wq, bq = np.asarray(inputs["Wq"], np.float32), np.asarray(inputs["bq"], np.float32)
    Wk, bk = np.asarray(inputs["Wk"], np.float32), np.asarray(inputs["bk"], np.float32)
    Wv, bv = np.asarray(inputs["Wv"], np.float32), np.asarray(inputs["bv"], np.float32)
    Wo, bo = np.asarray(inputs["Wo"], np.float32), np.asarray(inputs["bo"], np.float32)

    if not (mask > 0).all():
        return _reference_fallback(query, key_, value, mask,
                                   Wq, bq, Wk, bk, Wv, bv, Wo, bo)

    include_bias = bool(np.any(bq) or np.any(bk) or np.any(bv))
    nc = _get_nc(include_bias)
    if include_bias:
        in_maps = make_in_maps(query, key_, value, Wq, Wk, Wv, Wo, bq, bk, bv)
    else:
        in_maps = make_in_maps(query, key_, value, Wq, Wk, Wv, Wo)

    res = run_bass_kernel_spmd(nc, in_maps, core_ids=list(range(NCORES)))
    total = res.results[0]["out"]
    for c in range(1, NCORES):
        total = total + res.results[c]["out"]
    return (total + bo).astype(np.float32)
